# revision 57
# baseline (speedup 1.0000x reference)
"""Multi-head attention (B=2, S=2048, D=1024, H=16) on 8 Trainium2 cores.

Sharding: core = 4*b + g  (b = batch 0..1, g = head-group 0..3, 4 heads each).
Host sums the 4 per-group output partials per batch and adds b_o.

v2 schedule: the softmax exp stream on the ACT engine (~129us busy) is the
hard floor; the PE matmul total (~130us warm) nearly equals it.  Emission is
slot-based: one slot per (qc, pair, g) scores+exp group, and all other PE
work (projections, attn*V, out-projection) is list-scheduled into the slots
by deadline so the exp stream runs dense from ~16us and the PE never idles
long enough for the HAM clock gate to re-throttle.

  - DMA issue order is latency-critical-first (wq half 0, xq block 0,
    wk half 0, first 256 k-columns), no SBUF buffer reuse on input tiles so
    the in-order sync queue never stalls on WAR dependencies
  - Q/K projections in fp8e4 DoubleRow (contraction 256, ~2x PE rate and
    half the xq/xk DMA bytes; measured end-to-end rel err 1.45e-2 < 2e-2)
  - scores in bf16, two heads row-tiled concurrently in the PE halves
  - exp: ACT, [128,2,512] PSUM -> bf16 SBUF tiles (scale folded via scale=)
  - attn*V (U): bf16 matmuls with an appended ones-column accumulating the
    softmax denominator into row 64; deferred behind a deep PT tile pool so
    the first-pass K projections fit the early slots
  - normalize: reciprocal_approx_fast + gpsimd partition broadcast
  - out-projection: per-128-query-tile units as PE filler; tail units
    evacuate PSUM via the (then idle) ACT engine instead of DVE
"""

import os
from contextlib import ExitStack

import ml_dtypes
import numpy as np

import concourse.bass as bass
import concourse.tile as tile
from concourse import bacc, mybir

B, S, D = 2, 2048, 1024
H, DH = 16, 64
NCORES = 8
NG = 4                  # head-group shards
DG = D // NG            # 256 dims per head-group (4 heads)
P = 128
QC = 512                # q-chunk width
NQC = S // QC           # 4
NKT = S // P            # 16 k-tiles of 128
NKG = NKT // 2          # 8 k-groups of 256 (two 128-tiles)
CD = D // P             # 8 contraction tiles for the projections
F32 = mybir.dt.float32
BF16 = mybir.dt.bfloat16
FP8 = mybir.dt.float8e4
DR = mybir.MatmulPerfMode.DoubleRow
AF = mybir.ActivationFunctionType
SCALE = 1.0 / float(np.sqrt(D))

PT_BUFS = 28            # deep pool: U lags exp by up to ~12 slots
_TAGS = {}


def _body(ctx: ExitStack, tc: "tile.TileContext", io: dict):
    nc = tc.nc
    ctx.enter_context(nc.allow_low_precision(reason="bf16 matmul pipeline"))
    sb = ctx.enter_context(tc.tile_pool(name="sb", bufs=1))
    ps = ctx.enter_context(tc.tile_pool(name="ps", bufs=1, space="PSUM"))

    def MM(tag_, *a, **kw):
        mm = nc.tensor.matmul(*a, **kw)
        try:
            _TAGS[mm.ins.name] = tag_
        except Exception:
            pass
        return mm

    # ---- SBUF tiles (inputs fully resident; no WAR reuse on input DMAs).
    # xq/xk and wq/wk are fp8e4 in DoubleRow-interleaved layout
    # [..., c2, ko, .]: contraction row = c2*256 + ko*128 + partition.
    wq = sb.tile([P, 2, 4, 2, P], FP8, tag="wq", bufs=1, name="wq")
    wk = sb.tile([P, 2, 4, 2, P], FP8, tag="wk", bufs=1, name="wk")
    wv = sb.tile([P, CD, DG], BF16, tag="wv", bufs=1, name="wv")
    bq = sb.tile([P, 2], F32, tag="bq", bufs=1, name="bq")
    bk = sb.tile([P, 2], F32, tag="bk", bufs=1, name="bk")
    bvb = sb.tile([P, DG], BF16, tag="bvb", bufs=1, name="bvb")
    ones8 = sb.tile([P, 8], BF16, tag="ones8", bufs=1, name="ones8")
    xq = sb.tile([P, NQC, 4, 2, QC], FP8, tag="xq", bufs=1, name="xq")
    xkb = [sb.tile([P, 4, 2, QC], FP8, tag="xk", bufs=NQC, name=f"xk{b}") for b in range(NQC)]
    xvb = [sb.tile([P, CD, QC], BF16, tag="xv", bufs=NQC, name=f"xv{b}") for b in range(NQC)]
    woT = []
    for pr in range(2):
        woT.append(sb.tile([P, D], BF16, tag="wo", bufs=2, name=f"woT{pr}"))

    # warm the PE HAM clock gate + preload the exp table set immediately:
    # junk reads of an uninitialized scratch tile, results discarded.
    junk = sb.tile([P, QC], BF16, tag="junk", bufs=1, name="junk")
    nc.gpsimd.memset(junk[:], 0)
    warm = sb.tile([1, 8], F32, tag="warm", bufs=1, name="warm")
    nc.scalar.activation(warm[:], junk[0:1, 0:8], AF.Exp, scale=0.0)
    wps = ps.tile([P, QC], F32, tag="aux", bufs=2, name="warmps")
    for i in range(8):
        MM("warmup", wps[:], junk[:, 0:P], junk[:], start=(i == 0), stop=(i == 7))

    # ---- DMA issues, latency-critical first (~0.65us sync issue each).
    # HBM saturates at ~0.37 MB/us from ~8.3us; completion(cum MB) ~=
    # 8.3 + cum/0.37 us.  Tiny tensors (bq/bk) must sit early: queue FIFO
    # order means they land at their cumulative-byte position.
    nc.sync.dma_start(wq[:, 0], io["wq"][:, 0])              # 0.125 MB
    nc.sync.dma_start(xq[:, 0], io["xq"][:, 0])              # 0.625
    nc.sync.dma_start(wk[:, 0], io["wk"][:, 0])              # 0.75
    nc.sync.dma_start(xkb[0][:, :, :, 0:256], io["xk"][:, 0, :, :, 0:256])   # 1.0
    nc.sync.dma_start(bq[:], io["bq"][:])
    nc.sync.dma_start(bk[:], io["bk"][:])
    nc.sync.dma_start(xkb[0][:, :, :, 256:512], io["xk"][:, 0, :, :, 256:512])  # 1.25
    nc.sync.dma_start(wv[:], io["wv"][:])                    # 1.75
    nc.sync.dma_start(bvb[:], io["bvb"][:])
    nc.sync.dma_start(ones8[:], io["ones8"][:])
    nc.sync.dma_start(xkb[1][:], io["xk"][:, 1])             # 2.25
    nc.sync.dma_start(xvb[0][:], io["xv"][:, 0])             # 3.25 (early: V/U
    #   consumption starts in the otherwise DMA-starved early slots)
    nc.sync.dma_start(wq[:, 1], io["wq"][:, 1])              # 3.375
    nc.sync.dma_start(wk[:, 1], io["wk"][:, 1])              # 3.5
    nc.sync.dma_start(xkb[2][:], io["xk"][:, 2])             # 4.0
    nc.sync.dma_start(xkb[3][:], io["xk"][:, 3])             # 4.5
    nc.sync.dma_start(xvb[1][:], io["xv"][:, 1])             # 5.5
    nc.sync.dma_start(xvb[2][:], io["xv"][:, 2])             # 6.5
    nc.sync.dma_start(xvb[3][:], io["xv"][:, 3])             # 7.5
    nc.sync.dma_start(xq[:, 1], io["xq"][:, 1])              # 8.0
    for pr in range(2):
        nc.sync.dma_start(woT[pr][:], io["wo"][pr * P : (pr + 1) * P, :])  # 8.5
    nc.sync.dma_start(xq[:, 2], io["xq"][:, 2])              # 9.0
    nc.sync.dma_start(xq[:, 3], io["xq"][:, 3])              # 9.5

    # ---- emission helpers ---------------------------------------------------
    QT = {}            # (pr, qc) -> [128, 512] bf16
    KT = [None, None]  # pr -> [128, S] bf16
    for pr in range(2):
        KT[pr] = sb.tile([P, S], BF16, tag="kt", bufs=2, name=f"KT{pr}")
    VA = {}            # g -> [128, 2, 4, 65] bf16 (key, kk, head, dim+ones)
    PT = {}            # (qc, pair, g, kk) -> [128, 2, 512] bf16
    U = {}             # (qc, h) -> [65, 512] f32 psum
    UN = {}            # (qc, pair) -> [128, 512] bf16
    YSB = {}

    VAT = {}  # g -> tile (allocated at first half; published to VA when done)

    def emit_q_part(qc, pr):
        # full 512-query unit: 4 fp8 DoubleRow matmuls (contraction 256 each)
        psg = ps.tile([P, QC], F32, tag="aux", bufs=2, name=f"psq{qc}_{pr}")
        for c2 in range(4):
            MM("qproj", psg[:], wq[:, pr, c2], xq[:, qc, c2, :, :],
               start=(c2 == 0), stop=(c2 == 3), perf_mode=DR)
        QT[pr, qc] = sb.tile([P, QC], BF16, tag="qt", bufs=4, name=f"QT{qc}_{pr}")
        nc.vector.tensor_scalar_add(QT[pr, qc][:], psg[:], bq[:, pr : pr + 1])

    def emit_k_part(pr, gp, pc):
        # 256 k-columns [gp*512 + pc*256, ...): 4 fp8 DoubleRow matmuls
        lo, hi = pc * 256, (pc + 1) * 256
        psg = ps.tile([P, 256], F32, tag="aux", bufs=2, name=f"psk{pr}_{gp}_{pc}")
        for c2 in range(4):
            MM("kproj", psg[:], wk[:, pr, c2], xkb[gp][:, c2, :, lo:hi],
               start=(c2 == 0), stop=(c2 == 3), perf_mode=DR)
        nc.vector.tensor_scalar_add(
            KT[pr][:, gp * QC + lo : gp * QC + hi], psg[:], bk[:, pr : pr + 1]
        )

    def emit_v_half(g, j):
        # V rows for keys [g*256 + j*128, g*256 + (j+1)*128)
        psv = ps.tile([P, DG], F32, tag="aux", bufs=2, name=f"psv{g}_{j}")
        st_i = g * 2 + j
        for c in range(CD):
            MM("vproj", psv[:],
               xvb[st_i // 4][:, c, (st_i % 4) * P : (st_i % 4 + 1) * P],
               wv[:, c, :], start=(c == 0), stop=(c == CD - 1))
        if j == 0:
            VAT[g] = sb.tile([P, 2, 4, DH + 1], BF16, tag="va", bufs=NKG, name=f"VA{g}")
        vt = VAT[g]
        nc.vector.tensor_add(
            vt[:, j, :, 0:DH],
            psv[:].rearrange("p (h d) -> p h d", h=4),
            bvb[:].rearrange("p (h d) -> p h d", h=4),
        )
        if j == 1:
            nc.vector.tensor_copy(
                vt[:].rearrange("p a b c -> p (a b) c")[:, :, DH : DH + 1],
                ones8[:, :, None],
            )
            VA[g] = vt

    def emit_scores_exp(qc, pair, g):
        # One ST tile per (g, k-tile) holding BOTH heads of the pair; the two
        # matmuls hit different PE row-halves and run concurrently.
        pr = pair
        for kk in range(2):
            st = ps.tile([P, 2, QC], F32, tag="st", bufs=2, name=f"st{qc}_{pair}_{g}_{kk}")
            k_tile = g * 2 + kk
            for i, h in enumerate((2 * pair, 2 * pair + 1)):
                lo = (h % 2) * 64
                MM("scores", st[:, i, :],
                   KT[pr][lo : lo + 64, k_tile * P : (k_tile + 1) * P],
                   QT[pr, qc][lo : lo + 64, :],
                   start=True, stop=True, tile_position=(lo, 0))
            pt = sb.tile([P, 2, QC], BF16, tag="pt", bufs=PT_BUFS,
                         name=f"pt{qc}_{pair}_{g}_{kk}")
            nc.scalar.activation(pt[:], st[:], AF.Exp, scale=SCALE)
            PT[qc, pair, g, kk] = pt

    def emit_u_group(qc, pair, g):
        # all 4 U matmuls for one (qc, pair, g): 2 heads x 2 k-tiles;
        # kk-outer so consecutive matmuls alternate PSUM banks (h0/h1)
        for h in (2 * pair, 2 * pair + 1):
            if (qc, h) not in U:
                U[qc, h] = ps.tile([DH + 1, QC], F32, tag="u", bufs=2, name=f"U{qc}_{h}")
        for kk in range(2):
            for h in (2 * pair, 2 * pair + 1):
                MM("u", U[qc, h][:], VA[g][:, kk, h, 0 : DH + 1],
                   PT[qc, pair, g, kk][:, h % 2, :],
                   start=(g == 0 and kk == 0), stop=(g == NKG - 1 and kk == 1))
        del PT[qc, pair, g, 0], PT[qc, pair, g, 1]

    def emit_normalize(qc, pair):
        # UN rows 0..63 = head 2*pair, rows 64..127 = head 2*pair+1 (DMA shift)
        un = sb.tile([P, QC], BF16, tag="un", bufs=4, name=f"UN{qc}_{pair}")
        UN[qc, pair] = un
        # phase-ordered across both heads so the z0 DMAs issue back-to-back
        # and the per-head chains pipeline (tail latency critical)
        ucp_, rb_ = {}, {}
        heads = (2 * pair + 1, 2 * pair)
        for h in heads:
            u = U.pop((qc, h))
            # evacuate U from PSUM right away: the bank gates the next pair
            ucp = sb.tile([65, QC], F32, tag="ucp", bufs=2, name=f"ucp{qc}_{h}")
            nc.vector.tensor_copy(ucp[:], u[0:65, :])
            ucp_[h] = ucp
            z0 = sb.tile([1, QC], F32, tag="z0", bufs=2, name=f"z0_{qc}_{h}")
            nc.sync.dma_start(z0[:], ucp[64:65, :])
            rz = sb.tile([1, QC], F32, tag="rz", bufs=2, name=f"rz{qc}_{h}")
            nc.vector.reciprocal_approx_fast(rz[:], z0[:])
            rb_[h] = rz
        for h in heads:
            rb = sb.tile([64, QC], F32, tag="rb", bufs=2, name=f"rb{qc}_{h}")
            nc.gpsimd.partition_broadcast(rb[:], rb_[h][:], channels=64)
            if h % 2 == 0:
                nc.vector.tensor_mul(un[0:64, :], ucp_[h][0:64, :], rb[:])
            else:
                tmp = sb.tile([64, QC], BF16, tag="untmp", bufs=2, name=f"ut{qc}_{h}")
                nc.vector.tensor_mul(tmp[:], ucp_[h][0:64, :], rb[:])
                nc.sync.dma_start(un[64:128, :], tmp[:])

    YPS = {}

    def emit_outproj_pr0(qcp, qi, ec, ppool="aux"):
        qt = qcp * 4 + qi
        if qt not in YSB:
            YSB[qt] = sb.tile([P, D], BF16, tag="y", bufs=3, name=f"Y{qt}")
        if ppool == "st":
            stt = ps.tile([P, 2, QC], F32, tag="st", bufs=2, name=f"yp{qt}_{ec}")
            yps = stt[:, 0, :]
        else:
            yps = ps.tile([P, QC], F32, tag="aux", bufs=2, name=f"yp{qt}_{ec}")
        YPS[qt, ec] = yps
        MM("oproj", yps[:], UN[qcp, 0][:, qi * P : (qi + 1) * P],
           woT[0][:, ec * QC : (ec + 1) * QC], start=True, stop=False)

    def emit_outproj_pr1(qcp, qi, ec, evac="dve"):
        qt = qcp * 4 + qi
        yps = YPS.pop((qt, ec))
        ysb = YSB[qt]
        MM("oproj", yps[:], UN[qcp, 1][:, qi * P : (qi + 1) * P],
           woT[1][:, ec * QC : (ec + 1) * QC], start=False, stop=True)
        if evac == "act":
            nc.scalar.activation(ysb[:, ec * QC : (ec + 1) * QC], yps[:], AF.Copy)
        else:
            nc.vector.tensor_copy(ysb[:, ec * QC : (ec + 1) * QC], yps[:])
        if ec == 1:
            nc.sync.dma_start(io["y"][qt * P : (qt + 1) * P, :], ysb[:])

    def emit_outproj_unit(qcp, qi, ec, evac="dve", ppool="aux"):
        emit_outproj_pr0(qcp, qi, ec, ppool=ppool)
        emit_outproj_pr1(qcp, qi, ec, evac=evac)

    # ---- slot scheduler -----------------------------------------------------
    # One slot per (qc, pair, g) = ~2.0us of ACT.  Fill each slot's PE budget
    # with filler items by (ready, deadline); U groups drain FIFO with a lag
    # bounded by the PT pool.  ready/deadline are in slot units, from the DMA
    # completion model (slot s streams at ~16+2s us).
    def sl(t_us):  # us -> slot coordinate
        return (t_us - 13.0) / 2.0

    # Filler units, keyed so scores/U can FORCE their producers to emit first
    # (emission order is PE program order; a producer emitted after its
    # consumer would be a race).  ready = DMA-model availability (soft),
    # deadline = last slot at which it should go out (soft priority).
    filler = {}   # id -> dict(ready, deadline, cost, fn)

    def add(fid, ready_us, deadline_slot, cost, fn):
        filler[fid] = dict(ready=sl(ready_us), deadline=deadline_slot,
                           cost=cost, fn=fn)

    # Ready times (us) from the measured HBM completion curve:
    # land(cum MB) ~= 8.3 + cum/0.33, +0.8us margin.  A too-early emission
    # stalls the in-order PE FIFO and everything behind it, so these are
    # deliberately conservative.  All pieces are ~0.9us of PE so one piece
    # fits a slot's filler budget without opening an exp gap.
    # K piece (pr, gp, pc) covers scores g = 2*gp + pc of pair pr
    kland = {(0, 0, 1): 12.9, (0, 1, 0): 16.5, (0, 1, 1): 16.5,
             (0, 2, 0): 21.2, (0, 2, 1): 21.2, (0, 3, 0): 22.7, (0, 3, 1): 22.7,
             (1, 0, 0): 20.5, (1, 0, 1): 20.5, (1, 1, 0): 20.5, (1, 1, 1): 20.5,
             (1, 2, 0): 22.0, (1, 2, 1): 22.0, (1, 3, 0): 23.5, (1, 3, 1): 23.5}
    for (pr_, gp_, pc_), rdy in kland.items():
        add(("k", pr_, gp_, pc_), rdy, 8 * pr_ + 2 * gp_ + pc_ - 1.0, 0.7,
            lambda pr_=pr_, gp_=gp_, pc_=pc_: emit_k_part(pr_, gp_, pc_))
    # Q units (qc, pr): first used by scores slot 16*qc + 8*pr
    qland = {0: 22.0, 1: 34.1, 2: 37.2, 3: 38.7}
    for qc_ in range(4):
        for pr_ in range(2):
            if qc_ == 0 and pr_ == 0:
                continue
            add(("q", qc_, pr_), qland[qc_], 16 * qc_ + 8 * pr_ - 1.5,
                1.1, lambda qc_=qc_, pr_=pr_: emit_q_part(qc_, pr_))
    # V halves (g, j): VA[g] needs xv block g//2 (xv0 now lands ~17.5 so V/U
    # work fills the DMA-starved early slots)
    vland = {0: 19.0, 1: 25.7, 2: 28.8, 3: 31.8}
    for g_ in range(NKG):
        for j_ in range(2):
            add(("v", g_, j_), vland[g_ // 2], 7.0 + 1.6 * g_ + 0.5 * j_,
                0.9, lambda g_=g_, j_=j_: emit_v_half(g_, j_))

    state = dict(pt_live=0, spent=0.0)

    def force(fid):
        it = filler.pop(fid, None)
        if it is not None:
            it["fn"]()
            state["spent"] += it["cost"]

    u_q = []           # FIFO of (qc, pair, g) groups awaiting emission
    norm_pending = {}  # (qc, pair) -> groups left
    oproj_q = []
    jn = [0]

    def emit_junk(n):
        # HAM keep-warm filler: discarded matmuls into the (idle) u-pool bank
        for _ in range(n):
            jt = ps.tile([DH + 1, QC], F32, tag="u", bufs=2, name=f"jp{jn[0]}")
            jn[0] += 1
            MM("warmup", jt[:], junk[:, 0 : DH + 1], junk[:], start=True, stop=True)

    def emit_u_front():
        qc_, pair_, g_ = u_q.pop(0)
        force(("v", g_, 0))
        force(("v", g_, 1))
        emit_u_group(qc_, pair_, g_)
        state["pt_live"] -= 2
        state["spent"] += 0.9
        key = (qc_, pair_)
        norm_pending[key] -= 1
        if norm_pending[key] == 0:
            # UN pool is 4 deep: before rewriting UN(qc-2, pair)'s buffer,
            # every out-projection unit reading it must already be emitted
            while oproj_q and oproj_q[0][0] <= qc_ - 2:
                emit_outproj_unit(*oproj_q.pop(0))
                state["spent"] += 0.5
            emit_normalize(qc_, pair_)
            if pair_ == 1:
                oproj_q.extend((qc_, qi, ec) for qi in range(4) for ec in range(2))

    SLOTS = [(qc, pair, g) for qc in range(NQC) for pair in range(2)
             for g in range(NKG)]

    # pre-stream: Q(0,0) and the first K piece (g=0 only needs k-cols 0..255);
    # junk matmuls bridge the xk0a DMA wait so the clock stays warm
    emit_q_part(0, 0)
    emit_junk(2)
    emit_k_part(0, 0, 0)

    for s, (qc, pair, g) in enumerate(SLOTS):
        # force-drain U if the PT pool is nearly exhausted
        while state["pt_live"] >= PT_BUFS - 4 and u_q:
            emit_u_front()
        # taper the U lag near the end so the tail is short
        while len(u_q) > max(2, 62 - s):
            emit_u_front()
        # hard dependencies of this slot's scores
        force(("k", pair, g // 2, g % 2))
        force(("q", qc, pair))
        emit_scores_exp(qc, pair, g)
        state["pt_live"] += 2
        u_q.append((qc, pair, g))
        norm_pending[(qc, pair)] = norm_pending.get((qc, pair), 0) + 1

        budget = 1.55 - state["spent"]
        state["spent"] = 0.0
        # large U backlog: drain ahead of filler so the PT-pressure guard
        # never has to burst-drain (bursts open exp gaps)
        nu = 0
        while budget > 0 and nu < 2 and len(u_q) > 8 and u_q[0][2] in VA:
            emit_u_front()
            state["spent"] = 0.0
            budget -= 0.9
            nu += 1
        # urgent filler (deadline within 2 slots)
        while budget > -0.3:
            cand = [fid for fid, it in filler.items()
                    if it["ready"] <= s + 0.01 and it["deadline"] <= s + 2.0]
            if not cand:
                break
            fid = min(cand, key=lambda f: filler[f]["deadline"])
            budget -= filler[fid]["cost"]
            force(fid)
            state["spent"] = 0.0
        # U backlog (keep a minimum lag; cap the burst to avoid exp gaps)
        nu = 0
        while budget > 0 and nu < 2 and len(u_q) > 3 and u_q[0][2] in VA:
            emit_u_front()
            state["spent"] = 0.0
            budget -= 0.9
            nu += 1
        # out-projection filler
        while budget > 0 and oproj_q:
            emit_outproj_unit(*oproj_q.pop(0))
            budget -= 0.5
        # remaining filler by deadline even if not urgent; don't cram a unit
        # that clearly overflows the slot (the overflow becomes an exp gap)
        while budget > 0:
            cand = [fid for fid, it in filler.items() if it["ready"] <= s + 0.01]
            if not cand:
                break
            fid = min(cand, key=lambda f: filler[f]["deadline"])
            if budget < 0.6 * filler[fid]["cost"]:
                break
            budget -= filler[fid]["cost"]
            force(fid)
            state["spent"] = 0.0
        # DMA-starved early slots: keep the PE clock warm with junk.
        # Only while no real U tile is accumulating (junk shares the u pool).
        if s < 8 and budget > 0.6 and not U:
            emit_junk(2)

    # ---- tail: drain everything left ---------------------------------------
    for fid in sorted(filler, key=lambda f: filler[f]["deadline"]):
        force(fid)
    # units whose chunk is fully normalized already
    while oproj_q and oproj_q[0][0] <= NQC - 2:
        emit_outproj_unit(*oproj_q.pop(0))
    # Last chunk's units: norm(3,1) only fires inside the final U drain, so
    # build the list explicitly.  pr0 halves need only UN[3,0] (ready now)
    # and run as PE filler under the final U groups + normalize chain.
    tail_units = [(NQC - 1, qi, ec) for qi in range(4) for ec in range(2)]
    presplit = tail_units[:4]
    for n, (qcp_, qi_, ec_) in enumerate(presplit):
        emit_outproj_pr0(qcp_, qi_, ec_, ppool="st" if n % 2 == 0 else "aux")
    while u_q:
        emit_u_front()
    oproj_q.clear()  # norm(3,1) just queued the last chunk; emitted below
    # keep the PE busy (and the clock warm) under the final normalize chain
    # (~5.5us of filler: the chain ends ~1.3us after the last MULTIPLY)
    emit_junk(24)
    for n, (qcp_, qi_, ec_) in enumerate(presplit):
        emit_outproj_pr1(qcp_, qi_, ec_, evac="act" if n % 2 == 0 else "dve")
    for n, unit in enumerate(tail_units[4:]):
        emit_outproj_unit(*unit, evac="act" if n % 2 == 0 else "dve",
                          ppool="st" if n % 2 == 0 else "aux")


def build_program():
    nc = bacc.Bacc(
        "TRN2", target_bir_lowering=False, debug=False, num_devices=NCORES
    )
    io = {
        "xq": nc.dram_tensor("xq", [P, NQC, 4, 2, QC], FP8, kind="ExternalInput").ap(),
        "xk": nc.dram_tensor("xk", [P, NQC, 4, 2, QC], FP8, kind="ExternalInput").ap(),
        "xv": nc.dram_tensor("xv", [P, NQC, CD, QC], BF16, kind="ExternalInput").ap(),
        "wq": nc.dram_tensor("wq", [P, 2, 4, 2, P], FP8, kind="ExternalInput").ap(),
        "wk": nc.dram_tensor("wk", [P, 2, 4, 2, P], FP8, kind="ExternalInput").ap(),
        "wv": nc.dram_tensor("wv", [P, CD, DG], BF16, kind="ExternalInput").ap(),
        "wo": nc.dram_tensor("wo", [DG, D], BF16, kind="ExternalInput").ap(),
        "bq": nc.dram_tensor("bq", [P, 2], F32, kind="ExternalInput").ap(),
        "bk": nc.dram_tensor("bk", [P, 2], F32, kind="ExternalInput").ap(),
        "bvb": nc.dram_tensor("bvb", [P, DG], BF16, kind="ExternalInput").ap(),
        "ones8": nc.dram_tensor("ones8", [P, 8], BF16, kind="ExternalInput").ap(),
        "y": nc.dram_tensor("y", [S, D], BF16, kind="ExternalOutput").ap(),
    }
    with tile.TileContext(nc) as tc:
        with ExitStack() as ctx:
            _body(ctx, tc, io)
    nc.compile()
    try:
        import json
        with open("/tmp/mha_tags.json", "w") as f:
            json.dump(_TAGS, f)
    except Exception:
        pass
    return nc


_CACHE = {}


def _get_program():
    if "nc" not in _CACHE:
        _CACHE["nc"] = build_program()
    return _CACHE["nc"]


def make_in_maps(inputs):
    q = np.asarray(inputs["query"], np.float32)
    k = np.asarray(inputs["key"], np.float32)
    v = np.asarray(inputs["value"], np.float32)
    W_q = np.asarray(inputs["W_q"], np.float32)
    W_k = np.asarray(inputs["W_k"], np.float32)
    W_v = np.asarray(inputs["W_v"], np.float32)
    W_o = np.asarray(inputs["W_o"], np.float32)
    b_q = np.asarray(inputs["b_q"], np.float32)
    b_k = np.asarray(inputs["b_k"], np.float32)
    b_v = np.asarray(inputs["b_v"], np.float32)

    bf = ml_dtypes.bfloat16
    f8 = ml_dtypes.float8_e4m3

    def xarr(x, b):
        # x[b].T [D, S] -> [P, NQC, CD, QC]; element (p, blk, c, q) =
        # xT[c*P + p, blk*QC + q]
        xT = x[b].T.reshape(CD, P, NQC, QC).transpose(1, 2, 0, 3)
        return np.ascontiguousarray(xT).astype(bf)

    def x8arr(x, b):
        # DoubleRow layout [P, NQC, 4, 2, QC]: element (p, blk, c2, ko, q) =
        # x[b].T[c2*256 + ko*128 + p, blk*512 + q]
        xT = x[b].T.reshape(4, 2, P, NQC, QC).transpose(2, 3, 0, 1, 4)
        return np.ascontiguousarray(xT).astype(f8)

    def warr2(W, sl):
        # DoubleRow layout [P, 2, 4, 2, 128]: element (p, pr, c2, ko, j) =
        # W[sl].T[c2*256 + ko*128 + p, pr*128 + j]
        wt = W[sl, :].T.reshape(4, 2, P, 2, P).transpose(2, 3, 0, 1, 4)
        return np.ascontiguousarray(wt).astype(f8)

    def warr(W, sl):
        wt = W[sl, :].T.reshape(CD, P, DG).transpose(1, 0, 2)
        return np.ascontiguousarray(wt).astype(bf)

    def barr(b, sl):
        return np.ascontiguousarray(b[sl].reshape(2, P).T)

    in_maps = []
    for core in range(NCORES):
        b, g = divmod(core, NG)
        sl = slice(g * DG, (g + 1) * DG)
        in_maps.append(
            {
                "xq": x8arr(q, b),
                "xk": x8arr(k, b),
                "xv": xarr(v, b),
                "wq": warr2(W_q, sl),
                "wk": warr2(W_k, sl),
                "wv": warr(W_v, sl),
                "wo": np.ascontiguousarray(W_o[:, sl].T).astype(bf),
                "bq": barr(b_q, sl),
                "bk": barr(b_k, sl),
                "bvb": np.tile(b_v[sl][None, :], (P, 1)).astype(bf),
                "ones8": np.ones((P, 8), bf),
            }
        )
    return in_maps


def kernel(**inputs):
    from concourse.bass_utils import run_bass_kernel_spmd

    nc = _get_program()
    in_maps = make_in_maps(inputs)
    trace = bool(int(os.environ.get("MHA_TRACE", "0")))
    res = run_bass_kernel_spmd(nc, in_maps, list(range(NCORES)), trace=trace)
    _CACHE["last_results"] = res

    b_o = np.asarray(inputs["b_o"], np.float32)
    out = np.zeros((B, S, D), np.float32)
    for core in range(NCORES):
        b = core // NG
        out[b] += res.results[core]["y"].astype(np.float32)
    out += b_o[None, None, :]
    return out


# revision 58
# speedup vs baseline: 1.0055x; 1.0055x over previous
"""Multi-head attention (B=2, S=2048, D=1024, H=16) on 8 Trainium2 cores.

Sharding: core = 4*b + g  (b = batch 0..1, g = head-group 0..3, 4 heads each).
Host sums the 4 per-group output partials per batch and adds b_o.

v2 schedule: the softmax exp stream on the ACT engine (~129us busy) is the
hard floor; the PE matmul total (~130us warm) nearly equals it.  Emission is
slot-based: one slot per (qc, pair, g) scores+exp group, and all other PE
work (projections, attn*V, out-projection) is list-scheduled into the slots
by deadline so the exp stream runs dense from ~16us and the PE never idles
long enough for the HAM clock gate to re-throttle.

  - DMA issue order is latency-critical-first (wq half 0, xq block 0,
    wk half 0, first 256 k-columns), no SBUF buffer reuse on input tiles so
    the in-order sync queue never stalls on WAR dependencies
  - Q/K projections in fp8e4 DoubleRow (contraction 256, ~2x PE rate and
    half the xq/xk DMA bytes; measured end-to-end rel err 1.45e-2 < 2e-2)
  - scores in bf16, two heads row-tiled concurrently in the PE halves
  - exp: ACT, [128,2,512] PSUM -> bf16 SBUF tiles (scale folded via scale=)
  - attn*V (U): bf16 matmuls with an appended ones-column accumulating the
    softmax denominator into row 64; deferred behind a deep PT tile pool so
    the first-pass K projections fit the early slots
  - normalize: reciprocal_approx_fast + gpsimd partition broadcast
  - out-projection: per-128-query-tile units as PE filler; tail units
    evacuate PSUM via the (then idle) ACT engine instead of DVE
"""

import os
from contextlib import ExitStack

import ml_dtypes
import numpy as np

import concourse.bass as bass
import concourse.tile as tile
from concourse import bacc, mybir

B, S, D = 2, 2048, 1024
H, DH = 16, 64
NCORES = 8
NG = 4                  # head-group shards
DG = D // NG            # 256 dims per head-group (4 heads)
P = 128
QC = 512                # q-chunk width
NQC = S // QC           # 4
NKT = S // P            # 16 k-tiles of 128
NKG = NKT // 2          # 8 k-groups of 256 (two 128-tiles)
CD = D // P             # 8 contraction tiles for the projections
F32 = mybir.dt.float32
BF16 = mybir.dt.bfloat16
FP8 = mybir.dt.float8e4
DR = mybir.MatmulPerfMode.DoubleRow
AF = mybir.ActivationFunctionType
SCALE = 1.0 / float(np.sqrt(D))

PT_BUFS = 28            # deep pool: U lags exp by up to ~12 slots
_TAGS = {}


def _body(ctx: ExitStack, tc: "tile.TileContext", io: dict):
    nc = tc.nc
    ctx.enter_context(nc.allow_low_precision(reason="bf16 matmul pipeline"))
    sb = ctx.enter_context(tc.tile_pool(name="sb", bufs=1))
    ps = ctx.enter_context(tc.tile_pool(name="ps", bufs=1, space="PSUM"))

    def MM(tag_, *a, **kw):
        mm = nc.tensor.matmul(*a, **kw)
        try:
            _TAGS[mm.ins.name] = tag_
        except Exception:
            pass
        return mm

    # ---- SBUF tiles (inputs fully resident; no WAR reuse on input DMAs).
    # xq/xk and wq/wk are fp8e4 in DoubleRow-interleaved layout
    # [..., c2, ko, .]: contraction row = c2*256 + ko*128 + partition.
    wq = sb.tile([P, 2, 4, 2, P], FP8, tag="wq", bufs=1, name="wq")
    wk = sb.tile([P, 2, 4, 2, P], FP8, tag="wk", bufs=1, name="wk")
    wv = sb.tile([P, CD, DG], BF16, tag="wv", bufs=1, name="wv")
    bq = sb.tile([P, 2], F32, tag="bq", bufs=1, name="bq")
    bk = sb.tile([P, 2], F32, tag="bk", bufs=1, name="bk")
    bvb = sb.tile([P, DG], BF16, tag="bvb", bufs=1, name="bvb")
    ones8 = sb.tile([P, 8], BF16, tag="ones8", bufs=1, name="ones8")
    xq = sb.tile([P, NQC, 4, 2, QC], FP8, tag="xq", bufs=1, name="xq")
    xkb = [sb.tile([P, 4, 2, QC], FP8, tag="xk", bufs=NQC, name=f"xk{b}") for b in range(NQC)]
    xvb = [sb.tile([P, CD, QC], BF16, tag="xv", bufs=NQC, name=f"xv{b}") for b in range(NQC)]
    woT = []
    for pr in range(2):
        woT.append(sb.tile([P, D], BF16, tag="wo", bufs=2, name=f"woT{pr}"))

    # warm the PE HAM clock gate + preload the exp table set immediately:
    # junk reads of an uninitialized scratch tile, results discarded.
    junk = sb.tile([P, QC], BF16, tag="junk", bufs=1, name="junk")
    nc.gpsimd.memset(junk[:], 0)
    warm = sb.tile([1, 8], F32, tag="warm", bufs=1, name="warm")
    nc.scalar.activation(warm[:], junk[0:1, 0:8], AF.Exp, scale=0.0)
    wps = ps.tile([P, QC], F32, tag="aux", bufs=2, name="warmps")
    for i in range(8):
        MM("warmup", wps[:], junk[:, 0:P], junk[:], start=(i == 0), stop=(i == 7))

    # ---- DMA issues, latency-critical first (~0.65us sync issue each).
    # HBM saturates at ~0.37 MB/us from ~8.3us; completion(cum MB) ~=
    # 8.3 + cum/0.37 us.  Tiny tensors (bq/bk) must sit early: queue FIFO
    # order means they land at their cumulative-byte position.
    nc.sync.dma_start(wq[:, 0], io["wq"][:, 0])              # 0.125 MB
    nc.sync.dma_start(xq[:, 0], io["xq"][:, 0])              # 0.625
    nc.sync.dma_start(wk[:, 0], io["wk"][:, 0])              # 0.75
    nc.sync.dma_start(xkb[0][:, :, :, 0:256], io["xk"][:, 0, :, :, 0:256])   # 1.0
    nc.sync.dma_start(bq[:], io["bq"][:])
    nc.sync.dma_start(bk[:], io["bk"][:])
    nc.sync.dma_start(xkb[0][:, :, :, 256:512], io["xk"][:, 0, :, :, 256:512])  # 1.25
    nc.sync.dma_start(wv[:], io["wv"][:])                    # 1.75
    nc.sync.dma_start(bvb[:], io["bvb"][:])
    nc.sync.dma_start(ones8[:], io["ones8"][:])
    nc.sync.dma_start(xkb[1][:], io["xk"][:, 1])             # 2.25
    nc.sync.dma_start(xvb[0][:], io["xv"][:, 0])             # 3.25 (early: V/U
    #   consumption starts in the otherwise DMA-starved early slots)
    nc.sync.dma_start(wq[:, 1], io["wq"][:, 1])              # 3.375
    nc.sync.dma_start(wk[:, 1], io["wk"][:, 1])              # 3.5
    nc.sync.dma_start(xkb[2][:], io["xk"][:, 2])             # 4.0
    nc.sync.dma_start(xkb[3][:], io["xk"][:, 3])             # 4.5
    nc.sync.dma_start(xvb[1][:], io["xv"][:, 1])             # 5.5
    nc.sync.dma_start(xvb[2][:], io["xv"][:, 2])             # 6.5
    nc.sync.dma_start(xvb[3][:], io["xv"][:, 3])             # 7.5
    nc.sync.dma_start(xq[:, 1], io["xq"][:, 1])              # 8.0
    for pr in range(2):
        nc.sync.dma_start(woT[pr][:], io["wo"][pr * P : (pr + 1) * P, :])  # 8.5
    nc.sync.dma_start(xq[:, 2], io["xq"][:, 2])              # 9.0
    nc.sync.dma_start(xq[:, 3], io["xq"][:, 3])              # 9.5

    # ---- emission helpers ---------------------------------------------------
    QT = {}            # (pr, qc) -> [128, 512] bf16
    KT = [None, None]  # pr -> [128, S] bf16
    for pr in range(2):
        KT[pr] = sb.tile([P, S], BF16, tag="kt", bufs=2, name=f"KT{pr}")
    VA = {}            # g -> [128, 2, 4, 65] bf16 (key, kk, head, dim+ones)
    PT = {}            # (qc, pair, g, kk) -> [128, 2, 512] bf16
    U = {}             # (qc, h) -> [65, 512] f32 psum
    UN = {}            # (qc, pair) -> [128, 512] bf16
    YSB = {}

    VAT = {}  # g -> tile (allocated at first half; published to VA when done)

    def emit_q_part(qc, pr):
        # full 512-query unit: 4 fp8 DoubleRow matmuls (contraction 256 each)
        psg = ps.tile([P, QC], F32, tag="aux", bufs=2, name=f"psq{qc}_{pr}")
        for c2 in range(4):
            MM("qproj", psg[:], wq[:, pr, c2], xq[:, qc, c2, :, :],
               start=(c2 == 0), stop=(c2 == 3), perf_mode=DR)
        QT[pr, qc] = sb.tile([P, QC], BF16, tag="qt", bufs=4, name=f"QT{qc}_{pr}")
        nc.vector.tensor_scalar_add(QT[pr, qc][:], psg[:], bq[:, pr : pr + 1])

    def emit_k_part(pr, gp, pc):
        # 256 k-columns [gp*512 + pc*256, ...): 4 fp8 DoubleRow matmuls
        lo, hi = pc * 256, (pc + 1) * 256
        psg = ps.tile([P, 256], F32, tag="aux", bufs=2, name=f"psk{pr}_{gp}_{pc}")
        for c2 in range(4):
            MM("kproj", psg[:], wk[:, pr, c2], xkb[gp][:, c2, :, lo:hi],
               start=(c2 == 0), stop=(c2 == 3), perf_mode=DR)
        nc.vector.tensor_scalar_add(
            KT[pr][:, gp * QC + lo : gp * QC + hi], psg[:], bk[:, pr : pr + 1]
        )

    def emit_v_half(g, j):
        # V rows for keys [g*256 + j*128, g*256 + (j+1)*128)
        psv = ps.tile([P, DG], F32, tag="aux", bufs=2, name=f"psv{g}_{j}")
        st_i = g * 2 + j
        for c in range(CD):
            MM("vproj", psv[:],
               xvb[st_i // 4][:, c, (st_i % 4) * P : (st_i % 4 + 1) * P],
               wv[:, c, :], start=(c == 0), stop=(c == CD - 1))
        if j == 0:
            VAT[g] = sb.tile([P, 2, 4, DH + 1], BF16, tag="va", bufs=NKG, name=f"VA{g}")
        vt = VAT[g]
        nc.vector.tensor_add(
            vt[:, j, :, 0:DH],
            psv[:].rearrange("p (h d) -> p h d", h=4),
            bvb[:].rearrange("p (h d) -> p h d", h=4),
        )
        if j == 1:
            nc.vector.tensor_copy(
                vt[:].rearrange("p a b c -> p (a b) c")[:, :, DH : DH + 1],
                ones8[:, :, None],
            )
            VA[g] = vt

    def emit_scores_exp(qc, pair, g):
        # One ST tile per (g, k-tile) holding BOTH heads of the pair; the two
        # matmuls hit different PE row-halves and run concurrently.
        pr = pair
        for kk in range(2):
            st = ps.tile([P, 2, QC], F32, tag="st", bufs=2, name=f"st{qc}_{pair}_{g}_{kk}")
            k_tile = g * 2 + kk
            for i, h in enumerate((2 * pair, 2 * pair + 1)):
                lo = (h % 2) * 64
                MM("scores", st[:, i, :],
                   KT[pr][lo : lo + 64, k_tile * P : (k_tile + 1) * P],
                   QT[pr, qc][lo : lo + 64, :],
                   start=True, stop=True, tile_position=(lo, 0))
            pt = sb.tile([P, 2, QC], BF16, tag="pt", bufs=PT_BUFS,
                         name=f"pt{qc}_{pair}_{g}_{kk}")
            nc.scalar.activation(pt[:], st[:], AF.Exp, scale=SCALE)
            PT[qc, pair, g, kk] = pt

    def emit_u_group(qc, pair, g):
        # all 4 U matmuls for one (qc, pair, g): 2 heads x 2 k-tiles;
        # kk-outer so consecutive matmuls alternate PSUM banks (h0/h1)
        for h in (2 * pair, 2 * pair + 1):
            if (qc, h) not in U:
                U[qc, h] = ps.tile([DH + 1, QC], F32, tag="u", bufs=2, name=f"U{qc}_{h}")
        for kk in range(2):
            for h in (2 * pair, 2 * pair + 1):
                MM("u", U[qc, h][:], VA[g][:, kk, h, 0 : DH + 1],
                   PT[qc, pair, g, kk][:, h % 2, :],
                   start=(g == 0 and kk == 0), stop=(g == NKG - 1 and kk == 1))
        del PT[qc, pair, g, 0], PT[qc, pair, g, 1]

    def emit_normalize(qc, pair):
        # UN rows 0..63 = head 2*pair, rows 64..127 = head 2*pair+1 (DMA shift)
        un = sb.tile([P, QC], BF16, tag="un", bufs=4, name=f"UN{qc}_{pair}")
        UN[qc, pair] = un
        # phase-ordered across both heads so the z0 DMAs issue back-to-back
        # and the per-head chains pipeline (tail latency critical)
        ucp_, rb_ = {}, {}
        heads = (2 * pair + 1, 2 * pair)
        for h in heads:
            u = U.pop((qc, h))
            # evacuate U from PSUM right away: the bank gates the next pair
            ucp = sb.tile([65, QC], F32, tag="ucp", bufs=2, name=f"ucp{qc}_{h}")
            nc.vector.tensor_copy(ucp[:], u[0:65, :])
            ucp_[h] = ucp
            z0 = sb.tile([1, QC], F32, tag="z0", bufs=2, name=f"z0_{qc}_{h}")
            nc.sync.dma_start(z0[:], ucp[64:65, :])
            rz = sb.tile([1, QC], F32, tag="rz", bufs=2, name=f"rz{qc}_{h}")
            nc.vector.reciprocal_approx_fast(rz[:], z0[:])
            rb_[h] = rz
        for h in heads:
            rb = sb.tile([64, QC], F32, tag="rb", bufs=2, name=f"rb{qc}_{h}")
            nc.gpsimd.partition_broadcast(rb[:], rb_[h][:], channels=64)
            if h % 2 == 0:
                nc.vector.tensor_mul(un[0:64, :], ucp_[h][0:64, :], rb[:])
            else:
                tmp = sb.tile([64, QC], BF16, tag="untmp", bufs=2, name=f"ut{qc}_{h}")
                nc.vector.tensor_mul(tmp[:], ucp_[h][0:64, :], rb[:])
                nc.sync.dma_start(un[64:128, :], tmp[:])

    YPS = {}

    def emit_outproj_pr0(qcp, qi, ec, ppool="aux"):
        qt = qcp * 4 + qi
        if qt not in YSB:
            YSB[qt] = sb.tile([P, D], BF16, tag="y", bufs=3, name=f"Y{qt}")
        if ppool == "st":
            stt = ps.tile([P, 2, QC], F32, tag="st", bufs=2, name=f"yp{qt}_{ec}")
            yps = stt[:, 0, :]
        else:
            yps = ps.tile([P, QC], F32, tag="aux", bufs=2, name=f"yp{qt}_{ec}")
        YPS[qt, ec] = yps
        MM("oproj", yps[:], UN[qcp, 0][:, qi * P : (qi + 1) * P],
           woT[0][:, ec * QC : (ec + 1) * QC], start=True, stop=False)

    def emit_outproj_pr1(qcp, qi, ec, evac="dve"):
        qt = qcp * 4 + qi
        yps = YPS.pop((qt, ec))
        ysb = YSB[qt]
        MM("oproj", yps[:], UN[qcp, 1][:, qi * P : (qi + 1) * P],
           woT[1][:, ec * QC : (ec + 1) * QC], start=False, stop=True)
        if evac == "act":
            nc.scalar.activation(ysb[:, ec * QC : (ec + 1) * QC], yps[:], AF.Copy)
        else:
            nc.vector.tensor_copy(ysb[:, ec * QC : (ec + 1) * QC], yps[:])
        if ec == 1:
            nc.sync.dma_start(io["y"][qt * P : (qt + 1) * P, :], ysb[:])

    def emit_outproj_unit(qcp, qi, ec, evac="dve", ppool="aux"):
        emit_outproj_pr0(qcp, qi, ec, ppool=ppool)
        emit_outproj_pr1(qcp, qi, ec, evac=evac)

    # ---- slot scheduler -----------------------------------------------------
    # One slot per (qc, pair, g) = ~2.0us of ACT.  Fill each slot's PE budget
    # with filler items by (ready, deadline); U groups drain FIFO with a lag
    # bounded by the PT pool.  ready/deadline are in slot units, from the DMA
    # completion model (slot s streams at ~16+2s us).
    def sl(t_us):  # us -> slot coordinate
        return (t_us - 13.0) / 2.0

    # Filler units, keyed so scores/U can FORCE their producers to emit first
    # (emission order is PE program order; a producer emitted after its
    # consumer would be a race).  ready = DMA-model availability (soft),
    # deadline = last slot at which it should go out (soft priority).
    filler = {}   # id -> dict(ready, deadline, cost, fn)

    def add(fid, ready_us, deadline_slot, cost, fn):
        filler[fid] = dict(ready=sl(ready_us), deadline=deadline_slot,
                           cost=cost, fn=fn)

    # Ready times (us) from the measured HBM completion curve:
    # land(cum MB) ~= 8.3 + cum/0.33, +0.8us margin.  A too-early emission
    # stalls the in-order PE FIFO and everything behind it, so these are
    # deliberately conservative.  All pieces are ~0.9us of PE so one piece
    # fits a slot's filler budget without opening an exp gap.
    # K piece (pr, gp, pc) covers scores g = 2*gp + pc of pair pr
    kland = {(0, 0, 1): 12.9, (0, 1, 0): 16.5, (0, 1, 1): 16.5,
             (0, 2, 0): 21.2, (0, 2, 1): 21.2, (0, 3, 0): 22.7, (0, 3, 1): 22.7,
             (1, 0, 0): 20.5, (1, 0, 1): 20.5, (1, 1, 0): 20.5, (1, 1, 1): 20.5,
             (1, 2, 0): 22.0, (1, 2, 1): 22.0, (1, 3, 0): 23.5, (1, 3, 1): 23.5}
    for (pr_, gp_, pc_), rdy in kland.items():
        add(("k", pr_, gp_, pc_), rdy, 8 * pr_ + 2 * gp_ + pc_ - 1.0, 0.7,
            lambda pr_=pr_, gp_=gp_, pc_=pc_: emit_k_part(pr_, gp_, pc_))
    # Q units (qc, pr): first used by scores slot 16*qc + 8*pr
    qland = {0: 22.0, 1: 34.1, 2: 37.2, 3: 38.7}
    for qc_ in range(4):
        for pr_ in range(2):
            if qc_ == 0 and pr_ == 0:
                continue
            add(("q", qc_, pr_), qland[qc_], 16 * qc_ + 8 * pr_ - 1.5,
                1.1, lambda qc_=qc_, pr_=pr_: emit_q_part(qc_, pr_))
    # V halves (g, j): VA[g] needs xv block g//2 (xv0 now lands ~17.5 so V/U
    # work fills the DMA-starved early slots)
    vland = {0: 19.0, 1: 25.7, 2: 28.8, 3: 31.8}
    for g_ in range(NKG):
        for j_ in range(2):
            add(("v", g_, j_), vland[g_ // 2], 7.0 + 1.6 * g_ + 0.5 * j_,
                0.9, lambda g_=g_, j_=j_: emit_v_half(g_, j_))

    state = dict(pt_live=0, spent=0.0)

    def force(fid):
        it = filler.pop(fid, None)
        if it is not None:
            it["fn"]()
            state["spent"] += it["cost"]

    u_q = []           # FIFO of (qc, pair, g) groups awaiting emission
    norm_pending = {}  # (qc, pair) -> groups left
    oproj_q = []
    jn = [0]

    def emit_junk(n):
        # HAM keep-warm filler: discarded matmuls into the (idle) u-pool bank
        for _ in range(n):
            jt = ps.tile([DH + 1, QC], F32, tag="u", bufs=2, name=f"jp{jn[0]}")
            jn[0] += 1
            MM("warmup", jt[:], junk[:, 0 : DH + 1], junk[:], start=True, stop=True)

    def emit_u_front():
        qc_, pair_, g_ = u_q.pop(0)
        force(("v", g_, 0))
        force(("v", g_, 1))
        emit_u_group(qc_, pair_, g_)
        state["pt_live"] -= 2
        state["spent"] += 0.9
        key = (qc_, pair_)
        norm_pending[key] -= 1
        if norm_pending[key] == 0:
            # UN pool is 4 deep: before rewriting UN(qc-2, pair)'s buffer,
            # every out-projection unit reading it must already be emitted
            while oproj_q and oproj_q[0][0] <= qc_ - 2:
                emit_outproj_unit(*oproj_q.pop(0))
                state["spent"] += 0.5
            emit_normalize(qc_, pair_)
            if pair_ == 1:
                oproj_q.extend((qc_, qi, ec) for qi in range(4) for ec in range(2))

    SLOTS = [(qc, pair, g) for qc in range(NQC) for pair in range(2)
             for g in range(NKG)]

    # pre-stream: Q(0,0) and the first K piece (g=0 only needs k-cols 0..255);
    # junk matmuls bridge the xk0a DMA wait so the clock stays warm
    emit_q_part(0, 0)
    emit_junk(2)
    emit_k_part(0, 0, 0)

    for s, (qc, pair, g) in enumerate(SLOTS):
        # force-drain U if the PT pool is nearly exhausted
        while state["pt_live"] >= PT_BUFS - 4 and u_q:
            emit_u_front()
        # taper the U lag near the end so the tail is short
        while len(u_q) > max(2, 62 - s):
            emit_u_front()
        # hard dependencies of this slot's scores
        force(("k", pair, g // 2, g % 2))
        force(("q", qc, pair))
        emit_scores_exp(qc, pair, g)
        state["pt_live"] += 2
        u_q.append((qc, pair, g))
        norm_pending[(qc, pair)] = norm_pending.get((qc, pair), 0) + 1

        budget = 1.55 - state["spent"]
        state["spent"] = 0.0
        # urgent filler (deadline within 2 slots)
        while budget > -0.3:
            cand = [fid for fid, it in filler.items()
                    if it["ready"] <= s + 0.01 and it["deadline"] <= s + 2.0]
            if not cand:
                break
            fid = min(cand, key=lambda f: filler[f]["deadline"])
            budget -= filler[fid]["cost"]
            force(fid)
            state["spent"] = 0.0
        # U backlog (keep a minimum lag; cap the burst to avoid exp gaps)
        nu = 0
        while budget > 0 and nu < 2 and len(u_q) > 3 and u_q[0][2] in VA:
            emit_u_front()
            state["spent"] = 0.0
            budget -= 0.9
            nu += 1
        # out-projection filler
        while budget > 0 and oproj_q:
            emit_outproj_unit(*oproj_q.pop(0))
            budget -= 0.5
        # remaining filler by deadline even if not urgent; don't cram a unit
        # that clearly overflows the slot (the overflow becomes an exp gap)
        while budget > 0:
            cand = [fid for fid, it in filler.items() if it["ready"] <= s + 0.01]
            if not cand:
                break
            fid = min(cand, key=lambda f: filler[f]["deadline"])
            if budget < 0.6 * filler[fid]["cost"]:
                break
            budget -= filler[fid]["cost"]
            force(fid)
            state["spent"] = 0.0
        # DMA-starved early slots: keep the PE clock warm with junk.
        # Only while no real U tile is accumulating (junk shares the u pool).
        if s < 8 and budget > 0.6 and not U:
            emit_junk(2)

    # ---- tail: drain everything left ---------------------------------------
    for fid in sorted(filler, key=lambda f: filler[f]["deadline"]):
        force(fid)
    # units whose chunk is fully normalized already
    while oproj_q and oproj_q[0][0] <= NQC - 2:
        emit_outproj_unit(*oproj_q.pop(0))
    # Last chunk's units: norm(3,1) only fires inside the final U drain, so
    # build the list explicitly.  pr0 halves need only UN[3,0] (ready now)
    # and run as PE filler under the final U groups + normalize chain.
    tail_units = [(NQC - 1, qi, ec) for qi in range(4) for ec in range(2)]
    presplit = tail_units[:4]
    for n, (qcp_, qi_, ec_) in enumerate(presplit):
        emit_outproj_pr0(qcp_, qi_, ec_, ppool="st" if n % 2 == 0 else "aux")
    while u_q:
        emit_u_front()
    oproj_q.clear()  # norm(3,1) just queued the last chunk; emitted below
    # keep the PE busy (and the clock warm) under the final normalize chain
    # (~5.5us of filler: the chain ends ~1.3us after the last MULTIPLY)
    emit_junk(24)
    for n, (qcp_, qi_, ec_) in enumerate(presplit):
        emit_outproj_pr1(qcp_, qi_, ec_, evac="act" if n % 2 == 0 else "dve")
    for n, unit in enumerate(tail_units[4:]):
        emit_outproj_unit(*unit, evac="act" if n % 2 == 0 else "dve",
                          ppool="st" if n % 2 == 0 else "aux")


def build_program():
    nc = bacc.Bacc(
        "TRN2", target_bir_lowering=False, debug=False, num_devices=NCORES
    )
    io = {
        "xq": nc.dram_tensor("xq", [P, NQC, 4, 2, QC], FP8, kind="ExternalInput").ap(),
        "xk": nc.dram_tensor("xk", [P, NQC, 4, 2, QC], FP8, kind="ExternalInput").ap(),
        "xv": nc.dram_tensor("xv", [P, NQC, CD, QC], BF16, kind="ExternalInput").ap(),
        "wq": nc.dram_tensor("wq", [P, 2, 4, 2, P], FP8, kind="ExternalInput").ap(),
        "wk": nc.dram_tensor("wk", [P, 2, 4, 2, P], FP8, kind="ExternalInput").ap(),
        "wv": nc.dram_tensor("wv", [P, CD, DG], BF16, kind="ExternalInput").ap(),
        "wo": nc.dram_tensor("wo", [DG, D], BF16, kind="ExternalInput").ap(),
        "bq": nc.dram_tensor("bq", [P, 2], F32, kind="ExternalInput").ap(),
        "bk": nc.dram_tensor("bk", [P, 2], F32, kind="ExternalInput").ap(),
        "bvb": nc.dram_tensor("bvb", [P, DG], BF16, kind="ExternalInput").ap(),
        "ones8": nc.dram_tensor("ones8", [P, 8], BF16, kind="ExternalInput").ap(),
        "y": nc.dram_tensor("y", [S, D], BF16, kind="ExternalOutput").ap(),
    }
    with tile.TileContext(nc) as tc:
        with ExitStack() as ctx:
            _body(ctx, tc, io)
    nc.compile()
    try:
        import json
        with open("/tmp/mha_tags.json", "w") as f:
            json.dump(_TAGS, f)
    except Exception:
        pass
    return nc


_CACHE = {}


def _get_program():
    if "nc" not in _CACHE:
        _CACHE["nc"] = build_program()
    return _CACHE["nc"]


def make_in_maps(inputs):
    q = np.asarray(inputs["query"], np.float32)
    k = np.asarray(inputs["key"], np.float32)
    v = np.asarray(inputs["value"], np.float32)
    W_q = np.asarray(inputs["W_q"], np.float32)
    W_k = np.asarray(inputs["W_k"], np.float32)
    W_v = np.asarray(inputs["W_v"], np.float32)
    W_o = np.asarray(inputs["W_o"], np.float32)
    b_q = np.asarray(inputs["b_q"], np.float32)
    b_k = np.asarray(inputs["b_k"], np.float32)
    b_v = np.asarray(inputs["b_v"], np.float32)

    bf = ml_dtypes.bfloat16
    f8 = ml_dtypes.float8_e4m3

    def xarr(x, b):
        # x[b].T [D, S] -> [P, NQC, CD, QC]; element (p, blk, c, q) =
        # xT[c*P + p, blk*QC + q]
        xT = x[b].T.reshape(CD, P, NQC, QC).transpose(1, 2, 0, 3)
        return np.ascontiguousarray(xT).astype(bf)

    def x8arr(x, b):
        # DoubleRow layout [P, NQC, 4, 2, QC]: element (p, blk, c2, ko, q) =
        # x[b].T[c2*256 + ko*128 + p, blk*512 + q]
        xT = x[b].T.reshape(4, 2, P, NQC, QC).transpose(2, 3, 0, 1, 4)
        return np.ascontiguousarray(xT).astype(f8)

    def warr2(W, sl):
        # DoubleRow layout [P, 2, 4, 2, 128]: element (p, pr, c2, ko, j) =
        # W[sl].T[c2*256 + ko*128 + p, pr*128 + j]
        wt = W[sl, :].T.reshape(4, 2, P, 2, P).transpose(2, 3, 0, 1, 4)
        return np.ascontiguousarray(wt).astype(f8)

    def warr(W, sl):
        wt = W[sl, :].T.reshape(CD, P, DG).transpose(1, 0, 2)
        return np.ascontiguousarray(wt).astype(bf)

    def barr(b, sl):
        return np.ascontiguousarray(b[sl].reshape(2, P).T)

    in_maps = []
    for core in range(NCORES):
        b, g = divmod(core, NG)
        sl = slice(g * DG, (g + 1) * DG)
        in_maps.append(
            {
                "xq": x8arr(q, b),
                "xk": x8arr(k, b),
                "xv": xarr(v, b),
                "wq": warr2(W_q, sl),
                "wk": warr2(W_k, sl),
                "wv": warr(W_v, sl),
                "wo": np.ascontiguousarray(W_o[:, sl].T).astype(bf),
                "bq": barr(b_q, sl),
                "bk": barr(b_k, sl),
                "bvb": np.tile(b_v[sl][None, :], (P, 1)).astype(bf),
                "ones8": np.ones((P, 8), bf),
            }
        )
    return in_maps


def kernel(**inputs):
    from concourse.bass_utils import run_bass_kernel_spmd

    nc = _get_program()
    in_maps = make_in_maps(inputs)
    trace = bool(int(os.environ.get("MHA_TRACE", "0")))
    res = run_bass_kernel_spmd(nc, in_maps, list(range(NCORES)), trace=trace)
    _CACHE["last_results"] = res

    b_o = np.asarray(inputs["b_o"], np.float32)
    out = np.zeros((B, S, D), np.float32)
    for core in range(NCORES):
        b = core // NG
        out[b] += res.results[core]["y"].astype(np.float32)
    out += b_o[None, None, :]
    return out
